# revision 14
# baseline (speedup 1.0000x reference)
"""Fused multi-head attention (QKV proj + RoPE + causal softmax + out proj)
for Trainium2, sharded over 8 NeuronCores.

Sharding: data-parallel over batch (B=2) x tensor-parallel over heads
(16 heads -> 4 per core).  Each core computes, for its (batch, head-group):
  qT/kT = wq/wk^T-projections in [d, s] layout (f32r matmuls, fp32 PSUM)
  RoPE applied on-chip (DVE pair-swap via stream_shuffle + mul/add)
  scoresT[kp, q] = krot^T.T @ qrot (one K=128 matmul per tile)
  causal masking via a PE-accumulated triangular constant on diagonal tiles
  expT = exp(scale * scoresT) on ACT, f32r
  PV with a ones-augmented V column => unnormalized out + softmax denominator
  normalize (DVE reciprocal + tensor_scalar), PE-transpose to attnT[d, s]
  partial output y_g = attnT.T @ wo_rows  (summed over head-groups on host)

Inputs arrive full-size; host slices/transposes, feeds 8 SPMD cores, and
sums the 4 head-group partials per batch at the end.
"""

import math

import numpy as np

import concourse.bacc as bacc
import concourse.mybir as mybir
from concourse import tile
from concourse.bass_utils import run_bass_kernel_spmd

B, S, D, H = 2, 2048, 2048, 16
NCORES = 8
HG = 4  # heads per core
HD = D // H  # 128
DG = HG * HD  # 512 = per-core slice of D
P = 128
NKC = D // P  # 16 contraction chunks
SBLK = 512  # s-block width in projection passes
NSB = S // SBLK
NST = S // P  # 16 s-tiles of 128
QB = 512  # q-block width in attention
NQB = S // QB
EB = 512  # e-block width in out-projection
NEB = D // EB

F32R = mybir.dt.float32r
F32 = mybir.dt.float32
EXP = mybir.ActivationFunctionType.Exp
SCALE = 1.0 / math.sqrt(HD)
SWAP32 = [i ^ 1 for i in range(32)]
NEG = -1.0e9


def build_program(variant: str, dump: bool = False):
    """variant: 'causal' | 'none' | 'general'"""
    nc = bacc.Bacc("TRN2", target_bir_lowering=False, debug=False)
    xT = nc.dram_tensor("xT", [D, S], F32R, kind="ExternalInput")
    wq = nc.dram_tensor("wq", [D, DG], F32R, kind="ExternalInput")
    wk = nc.dram_tensor("wk", [D, DG], F32R, kind="ExternalInput")
    wv = nc.dram_tensor("wv", [D, DG], F32R, kind="ExternalInput")
    wo = nc.dram_tensor("wo", [DG, D], F32R, kind="ExternalInput")
    cosT = nc.dram_tensor("cosT", [HD, S], F32R, kind="ExternalInput")
    sinT = nc.dram_tensor("sinT", [HD, S], F32R, kind="ExternalInput")
    ident = nc.dram_tensor("ident", [P, P], F32R, kind="ExternalInput")
    tri = None
    maskT = None
    if variant == "causal":
        tri = nc.dram_tensor("tri", [P, P], F32R, kind="ExternalInput")
    elif variant == "general":
        # mask.T pre-scaled by sqrt(HD) on host so exp's scale recovers it
        maskT = nc.dram_tensor("maskT", [S, S], F32R, kind="ExternalInput")
    y = nc.dram_tensor("y", [S, D], F32, kind="ExternalOutput")
    d_qrot = d_krot = d_vaug = d_attnT = None
    if dump:
        d_qrot = nc.dram_tensor("d_qrot", [P, HG, S], F32, kind="ExternalOutput")
        d_krot = nc.dram_tensor("d_krot", [P, HG, S], F32, kind="ExternalOutput")
        d_vaug = nc.dram_tensor("d_vaug", [P, NST, HG, HD + 2], F32, kind="ExternalOutput")
        d_attnT = nc.dram_tensor("d_attnT", [P, HG, S], F32, kind="ExternalOutput")

    with tile.TileContext(nc) as tc:
        with (
            tc.tile_pool(name="const", bufs=1) as constp,
            tc.tile_pool(name="big", bufs=1) as bigp,
        ):
            tid = constp.tile([P, P], F32R)
            nc.sync.dma_start(tid[:], ident[:])
            ttri = None
            if variant == "causal":
                ttri = constp.tile([P, P], F32R)
                nc.sync.dma_start(ttri[:], tri[:])
            tcos = constp.tile([HD, S], F32R)
            nc.sync.dma_start(tcos[:], cosT[:])
            tsin = constp.tile([HD, S], F32R)
            nc.sync.dma_start(tsin[:], sinT[:])

            qrot = bigp.tile([P, HG, S], F32R, tag="qrot")
            krot = bigp.tile([P, HG, S], F32R, tag="krot")
            vaug = bigp.tile([P, NST, HG, HD + 2], F32R, tag="vaug")
            nc.vector.memset(vaug[:, :, :, HD : HD + 2].bitcast(F32), 1.0)

            # ---------------- projections + RoPE ----------------
            # One pass per projection; each streams xT once (kc-outer, so an
            # xT tile is consumed by the 4 in-flight accumulations then dies).
            with (
                tc.tile_pool(name="wpool", bufs=1) as wpool,
                tc.tile_pool(name="xpool", bufs=6) as xpool,
                tc.tile_pool(name="rope", bufs=3) as ropep,
                tc.tile_pool(name="pproj", bufs=8, space="PSUM") as pspool,
            ):
                for proj, wdram in (("q", wq), ("k", wk), ("v", wv)):
                    w_sb = wpool.tile([P, NKC, DG], F32R, tag="w", name=f"w_{proj}")
                    nc.sync.dma_start(
                        w_sb[:], wdram.ap().rearrange("(kc p) d -> p kc d", p=P)
                    )
                    for sb in range(NSB):
                        nun = SBLK // P if proj == "v" else HG
                        pss = [
                            pspool.tile(
                                [P, SBLK if proj != "v" else DG],
                                F32,
                                tag="proj",
                                name=f"ps_{proj}_{sb}_{u}",
                            )
                            for u in range(nun)
                        ]
                        for kc in range(NKC):
                            xt = xpool.tile([P, SBLK], F32R, tag="xt", name="xt")
                            nc.sync.dma_start(
                                xt[:],
                                xT[kc * P : (kc + 1) * P, sb * SBLK : (sb + 1) * SBLK],
                            )
                            if proj in ("q", "k"):
                                for dt in range(HG):
                                    nc.tensor.matmul(
                                        pss[dt][:],
                                        w_sb[:, kc, dt * HD : (dt + 1) * HD],
                                        xt[:],
                                        start=(kc == 0),
                                        stop=(kc == NKC - 1),
                                    )
                            else:
                                for st in range(SBLK // P):
                                    nc.tensor.matmul(
                                        pss[st][:],
                                        xt[:, st * P : (st + 1) * P],
                                        w_sb[:, kc, :],
                                        start=(kc == 0),
                                        stop=(kc == NKC - 1),
                                    )
                        if proj in ("q", "k"):
                            dstbuf = qrot if proj == "q" else krot
                            ssl = slice(sb * SBLK, (sb + 1) * SBLK)
                            for dt in range(HG):
                                ps = pss[dt]
                                tsw = ropep.tile([P, SBLK], F32, tag="tsw", name="tsw")
                                nc.vector.stream_shuffle(tsw[:], ps[:], SWAP32)
                                t1 = ropep.tile([P, SBLK], F32, tag="t1", name="t1")
                                nc.vector.tensor_mul(t1[:], ps[:], tcos[:, ssl])
                                t2 = ropep.tile([P, SBLK], F32, tag="t2", name="t2")
                                nc.vector.tensor_mul(t2[:], tsw[:], tsin[:, ssl])
                                nc.vector.tensor_add(
                                    dstbuf[:, dt, ssl], t1[:], t2[:]
                                )
                        else:
                            for st in range(SBLK // P):
                                st_g = sb * (SBLK // P) + st
                                for h in range(HG):
                                    nc.scalar.copy(
                                        vaug[:, st_g, h, 0:HD],
                                        pss[st][:, h * HD : (h + 1) * HD],
                                    )

            if dump:
                nc.sync.dma_start(d_qrot.ap(), qrot[:].bitcast(F32))
                nc.sync.dma_start(d_krot.ap(), krot[:].bitcast(F32))
                nc.sync.dma_start(d_vaug.ap(), vaug[:].bitcast(F32))

            # ---------------- attention ----------------
            with tc.tile_pool(name="attn_out", bufs=1) as atp:
              attnT = atp.tile([P, HG, S], F32R, tag="attnT")
              with (
                tc.tile_pool(name="mask", bufs=2) as maskp,
                tc.tile_pool(name="expp", bufs=4) as epool,
                tc.tile_pool(name="small", bufs=4) as smallp,
                tc.tile_pool(name="normp", bufs=3) as npool,
                tc.tile_pool(name="pscore", bufs=2, space="PSUM") as sppool,
                tc.tile_pool(name="paug", bufs=4, space="PSUM") as augpool,
                tc.tile_pool(name="ptr", bufs=2, space="PSUM") as trpool,
            ):
                for qb in range(NQB):
                    mts = None
                    if variant == "general":
                        mts = maskp.tile([P, NST, QB], F32R, tag="mt")
                        nc.sync.dma_start(
                            mts[:],
                            maskT[:, qb * QB : (qb + 1) * QB].rearrange(
                                "(kt p) q -> p kt q", p=P
                            ),
                        )
                    nkt = 4 * (qb + 1) if variant == "causal" else NST
                    for h in range(HG):
                        augs = [
                            augpool.tile([P, HD + 2], F32, tag="aug", name=f"aug{i}")
                            for i in range(4)
                        ]
                        for kt in range(nkt):
                            ps_s = sppool.tile([P, QB], F32, tag="scores")
                            if variant == "causal" and kt >= 4 * qb:
                                j = kt - 4 * qb
                                nc.tensor.matmul(
                                    ps_s[:, j * P : QB],
                                    krot[:, h, kt * P : (kt + 1) * P],
                                    qrot[:, h, qb * QB + j * P : (qb + 1) * QB],
                                    start=True,
                                    stop=False,
                                )
                                nc.tensor.matmul(
                                    ps_s[:, j * P : (j + 1) * P],
                                    tid[:],
                                    ttri[:],
                                    start=False,
                                    stop=True,
                                )
                                valid = slice(j * P, QB)
                                qt_lo = j
                            else:
                                last = variant != "general"
                                nc.tensor.matmul(
                                    ps_s[:],
                                    krot[:, h, kt * P : (kt + 1) * P],
                                    qrot[:, h, qb * QB : (qb + 1) * QB],
                                    start=True,
                                    stop=last,
                                )
                                if variant == "general":
                                    nc.tensor.matmul(
                                        ps_s[:],
                                        tid[:],
                                        mts[:, kt, :],
                                        start=False,
                                        stop=True,
                                    )
                                valid = slice(0, QB)
                                qt_lo = 0
                            texp = epool.tile([P, QB], F32R, tag="exp")
                            nc.scalar.activation(
                                texp[:, valid], ps_s[:, valid], EXP, scale=SCALE
                            )
                            for qt in range(qt_lo, 4):
                                if variant == "causal" and kt > 4 * qb + qt:
                                    continue
                                last_kt = 4 * qb + qt if variant == "causal" else NST - 1
                                nc.tensor.matmul(
                                    augs[qt][:],
                                    texp[:, qt * P : (qt + 1) * P],
                                    vaug[:, kt, h, :],
                                    start=(kt == 0),
                                    stop=(kt == last_kt),
                                )
                        for qt in range(4):
                            qt_g = qb * 4 + qt
                            rec = smallp.tile([P, 1], F32, tag="rec")
                            nc.vector.reciprocal(rec[:], augs[qt][:, HD : HD + 1])
                            attn_n = npool.tile([P, HD], F32R, tag="attn_n")
                            nc.vector.tensor_scalar_mul(
                                attn_n[:], augs[qt][:, 0:HD], rec[:]
                            )
                            ps_t = trpool.tile([P, P], F32R, tag="tr")
                            nc.tensor.transpose(ps_t[:], attn_n[:], tid[:])
                            nc.scalar.copy(
                                attnT[:, h, qt_g * P : (qt_g + 1) * P], ps_t[:]
                            )

              if dump:
                  nc.sync.dma_start(d_attnT.ap(), attnT[:].bitcast(F32))

              # ---------------- out projection ----------------
              with (
                tc.tile_pool(name="wopool", bufs=1) as wopool,
                tc.tile_pool(name="outp", bufs=4) as outp,
                tc.tile_pool(name="pout", bufs=4, space="PSUM") as opspool,
              ):
                wo_sb = wopool.tile([P, HG, D], F32R, tag="wo")
                nc.sync.dma_start(
                    wo_sb[:], wo.ap().rearrange("(dc p) e -> p dc e", p=P)
                )
                for st in range(NST):
                    for eb in range(NEB):
                        ps_o = opspool.tile([P, EB], F32, tag="out", name=f"o{st}_{eb}")
                        for dc in range(HG):
                            nc.tensor.matmul(
                                ps_o[:],
                                attnT[:, dc, st * P : (st + 1) * P],
                                wo_sb[:, dc, eb * EB : (eb + 1) * EB],
                                start=(dc == 0),
                                stop=(dc == HG - 1),
                            )
                        out_t = outp.tile([P, EB], F32, tag="outsb", name="outsb")
                        nc.scalar.copy(out_t[:], ps_o[:])
                        nc.sync.dma_start(
                            y[st * P : (st + 1) * P, eb * EB : (eb + 1) * EB], out_t[:]
                        )

    nc.compile()
    return nc


_PROGRAM_CACHE: dict[str, object] = {}
_last_in_maps = None


def _get_program(variant: str):
    if variant not in _PROGRAM_CACHE:
        _PROGRAM_CACHE[variant] = build_program(variant)
    return _PROGRAM_CACHE[variant]


def _detect_variant(mask: np.ndarray) -> str:
    if not np.any(mask):
        return "none"
    causal = np.triu(np.full((S, S), NEG, dtype=np.float32), 1)
    if np.array_equal(mask, causal):
        return "causal"
    return "general"


def kernel(x, wq, wk, wv, wo, cos, sin, mask):
    x = np.asarray(x, dtype=np.float32)
    wq = np.asarray(wq, dtype=np.float32)
    wk = np.asarray(wk, dtype=np.float32)
    wv = np.asarray(wv, dtype=np.float32)
    wo = np.asarray(wo, dtype=np.float32)
    cos = np.asarray(cos, dtype=np.float32)
    sin = np.asarray(sin, dtype=np.float32)
    mask = np.asarray(mask, dtype=np.float32)

    variant = _detect_variant(mask)
    nc = _get_program(variant)

    # host-side shared constants
    cosT = np.repeat(cos.T, 2, axis=0)  # [HD, S]
    sinT = np.repeat(sin.T, 2, axis=0)
    sinT[0::2, :] *= -1.0  # sign baked in: row 2i holds -sin, row 2i+1 holds +sin
    cosT = np.ascontiguousarray(cosT, dtype=np.float32)
    sinT = np.ascontiguousarray(sinT, dtype=np.float32)
    ident = np.eye(P, dtype=np.float32)
    shared = {"cosT": cosT, "sinT": sinT, "ident": ident}
    if variant == "causal":
        # scoresT layout is [kp, q]: masked where kp > q -> strict lower triangle
        shared["tri"] = np.tril(np.full((P, P), NEG, dtype=np.float32), -1)
    elif variant == "general":
        shared["maskT"] = np.ascontiguousarray(mask.T * math.sqrt(HD))

    xTs = [np.ascontiguousarray(x[b].T) for b in range(B)]
    in_maps = []
    for core in range(NCORES):
        b, g = divmod(core, NCORES // B)
        sl = slice(g * DG, (g + 1) * DG)
        in_maps.append(
            {
                "xT": xTs[b],
                "wq": np.ascontiguousarray(wq[:, sl]),
                "wk": np.ascontiguousarray(wk[:, sl]),
                "wv": np.ascontiguousarray(wv[:, sl]),
                "wo": np.ascontiguousarray(wo[sl, :]),
                **shared,
            }
        )

    global _last_in_maps
    _last_in_maps = in_maps

    res = run_bass_kernel_spmd(nc, in_maps, core_ids=list(range(NCORES)))

    out = np.empty((B, S, D), dtype=np.float32)
    gpb = NCORES // B
    for b in range(B):
        acc = np.zeros((S, D), dtype=np.float64)
        for g in range(gpb):
            acc += res.results[b * gpb + g]["y"].astype(np.float64)
        out[b] = acc.astype(np.float32)
    return out


# revision 17
# speedup vs baseline: 1.0441x; 1.0441x over previous
"""Fused multi-head attention (QKV proj + RoPE + causal softmax + out proj)
for Trainium2, sharded over 8 NeuronCores.

Sharding: data-parallel over batch (B=2) x tensor-parallel over heads
(16 heads -> 4 per core).  Each core computes, for its (batch, head-group):
  qT/kT = wq/wk^T-projections in [d, s] layout (CDT matmuls, fp32 PSUM)
  RoPE applied on-chip (DVE pair-swap via stream_shuffle + mul/add)
  scoresT[kp, q] = krot^T.T @ qrot (one K=128 matmul per tile)
  causal masking via a PE-accumulated triangular constant on diagonal tiles
  expT = exp(scale * scoresT) on ACT
  PV with a ones-augmented V column => unnormalized out + softmax denominator
  normalize (DVE reciprocal + tensor_scalar), PE-transpose to attnT[d, s]
  partial output y_g = attnT.T @ wo_rows  (summed over head-groups on host)

Inputs arrive full-size; host slices/transposes, feeds 8 SPMD cores, and
sums the 4 head-group partials per batch at the end.
"""

import math

import numpy as np

import concourse.bacc as bacc
import concourse.mybir as mybir
from concourse import tile
from concourse.bass_utils import run_bass_kernel_spmd

B, S, D, H = 2, 2048, 2048, 16
NCORES = 8
HG = 4  # heads per core
HD = D // H  # 128
DG = HG * HD  # 512 = per-core slice of D
P = 128
NKC = D // P  # 16 contraction chunks
SBLK = 512  # s-block width in projection passes
NSB = S // SBLK
NST = S // P  # 16 s-tiles of 128
QB = 512  # q-block width in attention
NQB = S // QB
EB = 512  # e-block width in out-projection
NEB = D // EB

F32 = mybir.dt.float32
EXP = mybir.ActivationFunctionType.Exp
SCALE = 1.0 / math.sqrt(HD)
SWAP32 = [i ^ 1 for i in range(32)]
NEG = -1.0e9

COMPUTE_DTYPE = "float32r"  # or "bfloat16"


def build_program(variant: str, dump: bool = False, cdt_name: str | None = None):
    """variant: 'causal' | 'none' | 'general'"""
    CDT = getattr(mybir.dt, cdt_name or COMPUTE_DTYPE)
    nc = bacc.Bacc("TRN2", target_bir_lowering=False, debug=False)
    xT = nc.dram_tensor("xT", [D, S], CDT, kind="ExternalInput")
    wq = nc.dram_tensor("wq", [D, DG], CDT, kind="ExternalInput")
    wk = nc.dram_tensor("wk", [D, DG], CDT, kind="ExternalInput")
    wv = nc.dram_tensor("wv", [D, DG], CDT, kind="ExternalInput")
    wo = nc.dram_tensor("wo", [DG, D], CDT, kind="ExternalInput")
    cosT = nc.dram_tensor("cosT", [HD, S], CDT, kind="ExternalInput")
    sinT = nc.dram_tensor("sinT", [HD, S], CDT, kind="ExternalInput")
    ident = nc.dram_tensor("ident", [P, P], CDT, kind="ExternalInput")
    tri = None
    maskT = None
    if variant == "causal":
        tri = nc.dram_tensor("tri", [P, P], CDT, kind="ExternalInput")
    elif variant == "general":
        # mask.T pre-scaled by sqrt(HD) on host so exp's scale recovers it
        maskT = nc.dram_tensor("maskT", [S, S], CDT, kind="ExternalInput")
    y = nc.dram_tensor("y", [S, D], F32, kind="ExternalOutput")
    d_qrot = d_krot = d_vaug = d_attnT = None
    if dump:
        d_qrot = nc.dram_tensor("d_qrot", [P, HG, S], CDT, kind="ExternalOutput")
        d_krot = nc.dram_tensor("d_krot", [P, HG, S], CDT, kind="ExternalOutput")
        d_vaug = nc.dram_tensor("d_vaug", [P, NST, HG, HD + 2], CDT, kind="ExternalOutput")
        d_attnT = nc.dram_tensor("d_attnT", [P, HG, S], CDT, kind="ExternalOutput")

    with tile.TileContext(nc) as tc:
        with (
            tc.tile_pool(name="const", bufs=1) as constp,
            tc.tile_pool(name="big", bufs=1) as bigp,
            # one PSUM pool shared by every phase: no pool-scoping barriers,
            # so attention matmuls can start while the v-pass drains.
            tc.tile_pool(name="ps", bufs=8, space="PSUM") as pspool,
        ):
            tid = constp.tile([P, P], CDT)
            nc.sync.dma_start(tid[:], ident[:])
            ttri = None
            if variant == "causal":
                ttri = constp.tile([P, P], CDT)
                nc.sync.dma_start(ttri[:], tri[:])
            tcos = constp.tile([HD, S], CDT)
            nc.sync.dma_start(tcos[:], cosT[:])
            tsin = constp.tile([HD, S], CDT)
            nc.sync.dma_start(tsin[:], sinT[:])

            qrot = bigp.tile([P, HG, S], CDT, tag="qrot")
            krot = bigp.tile([P, HG, S], CDT, tag="krot")
            vaug = bigp.tile([P, NST, HG, HD + 2], CDT, tag="vaug")
            ones_view = vaug[:, :, :, HD : HD + 2]
            if CDT == mybir.dt.float32r:
                nc.vector.memset(ones_view.bitcast(F32), 1.0)
            else:
                nc.vector.memset(ones_view, 1.0)

            # ---------------- projections + RoPE ----------------
            # Per-kc weight tiles stream ahead of the matmuls; xT row-block
            # tiles are consumed by the in-flight accumulations then die.
            with (
                tc.tile_pool(name="wpool", bufs=12) as wpool,
                tc.tile_pool(name="xpool", bufs=8) as xpool,
                tc.tile_pool(name="rope", bufs=3) as ropep,
            ):
                for proj, wdram in (("q", wq), ("k", wk), ("v", wv)):
                    for sb in range(NSB):
                        nun = SBLK // P if proj == "v" else HG
                        pss = [
                            pspool.tile(
                                [P, SBLK if proj != "v" else DG],
                                F32,
                                tag="ps",
                                name=f"ps_{proj}_{sb}_{u}",
                            )
                            for u in range(nun)
                        ]
                        for kc in range(NKC):
                            xt = xpool.tile([P, SBLK], CDT, tag="xt", name="xt")
                            nc.sync.dma_start(
                                xt[:],
                                xT[kc * P : (kc + 1) * P, sb * SBLK : (sb + 1) * SBLK],
                            )
                            wt = wpool.tile([P, DG], CDT, tag="wt", name="wt")
                            nc.sync.dma_start(wt[:], wdram[kc * P : (kc + 1) * P, :])
                            if proj in ("q", "k"):
                                for dt in range(HG):
                                    nc.tensor.matmul(
                                        pss[dt][:],
                                        wt[:, dt * HD : (dt + 1) * HD],
                                        xt[:],
                                        start=(kc == 0),
                                        stop=(kc == NKC - 1),
                                    )
                            else:
                                for st in range(SBLK // P):
                                    nc.tensor.matmul(
                                        pss[st][:],
                                        xt[:, st * P : (st + 1) * P],
                                        wt[:],
                                        start=(kc == 0),
                                        stop=(kc == NKC - 1),
                                    )
                        if proj in ("q", "k"):
                            dstbuf = qrot if proj == "q" else krot
                            ssl = slice(sb * SBLK, (sb + 1) * SBLK)
                            for dt in range(HG):
                                ps = pss[dt]
                                tsw = ropep.tile([P, SBLK], F32, tag="tsw", name="tsw")
                                nc.vector.stream_shuffle(tsw[:], ps[:], SWAP32)
                                t1 = ropep.tile([P, SBLK], F32, tag="t1", name="t1")
                                nc.vector.tensor_mul(t1[:], ps[:], tcos[:, ssl])
                                t2 = ropep.tile([P, SBLK], F32, tag="t2", name="t2")
                                nc.vector.tensor_mul(t2[:], tsw[:], tsin[:, ssl])
                                nc.vector.tensor_add(
                                    dstbuf[:, dt, ssl], t1[:], t2[:]
                                )
                        else:
                            for st in range(SBLK // P):
                                st_g = sb * (SBLK // P) + st
                                for h in range(HG):
                                    nc.scalar.copy(
                                        vaug[:, st_g, h, 0:HD],
                                        pss[st][:, h * HD : (h + 1) * HD],
                                    )

            if dump:
                nc.sync.dma_start(d_qrot.ap(), qrot[:])
                nc.sync.dma_start(d_krot.ap(), krot[:])
                nc.sync.dma_start(d_vaug.ap(), vaug[:])

            # ---------------- attention (+ wo weights prefetch) ----------------
            with (
                tc.tile_pool(name="attn_out", bufs=1) as atp,
                tc.tile_pool(name="wopool", bufs=1) as wopool,
            ):
              attnT = atp.tile([P, HG, S], CDT, tag="attnT")
              wo_sb = wopool.tile([P, HG, D], CDT, tag="wo")
              nc.sync.dma_start(
                  wo_sb[:], wo.ap().rearrange("(dc p) e -> p dc e", p=P)
              )
              with (
                tc.tile_pool(name="mask", bufs=2) as maskp,
                tc.tile_pool(name="expp", bufs=4) as epool,
                tc.tile_pool(name="small", bufs=4) as smallp,
                tc.tile_pool(name="normp", bufs=3) as npool,
              ):
                for qb in range(NQB):
                    mts = None
                    if variant == "general":
                        mts = maskp.tile([P, NST, QB], CDT, tag="mt", name="mt")
                        nc.sync.dma_start(
                            mts[:],
                            maskT[:, qb * QB : (qb + 1) * QB].rearrange(
                                "(kt p) q -> p kt q", p=P
                            ),
                        )
                    nkt = 4 * (qb + 1) if variant == "causal" else NST
                    for h in range(HG):
                        augs = [
                            pspool.tile([P, HD + 2], F32, tag="ps", name=f"aug{i}")
                            for i in range(4)
                        ]
                        for kt in range(nkt):
                            ps_s = pspool.tile([P, QB], F32, tag="ps", name="scores")
                            if variant == "causal" and kt >= 4 * qb:
                                j = kt - 4 * qb
                                nc.tensor.matmul(
                                    ps_s[:, j * P : QB],
                                    krot[:, h, kt * P : (kt + 1) * P],
                                    qrot[:, h, qb * QB + j * P : (qb + 1) * QB],
                                    start=True,
                                    stop=False,
                                )
                                nc.tensor.matmul(
                                    ps_s[:, j * P : (j + 1) * P],
                                    tid[:],
                                    ttri[:],
                                    start=False,
                                    stop=True,
                                )
                                valid = slice(j * P, QB)
                                qt_lo = j
                            else:
                                last = variant != "general"
                                nc.tensor.matmul(
                                    ps_s[:],
                                    krot[:, h, kt * P : (kt + 1) * P],
                                    qrot[:, h, qb * QB : (qb + 1) * QB],
                                    start=True,
                                    stop=last,
                                )
                                if variant == "general":
                                    nc.tensor.matmul(
                                        ps_s[:],
                                        tid[:],
                                        mts[:, kt, :],
                                        start=False,
                                        stop=True,
                                    )
                                valid = slice(0, QB)
                                qt_lo = 0
                            texp = epool.tile([P, QB], CDT, tag="exp", name="exp")
                            nc.scalar.activation(
                                texp[:, valid], ps_s[:, valid], EXP, scale=SCALE
                            )
                            for qt in range(qt_lo, 4):
                                if variant == "causal" and kt > 4 * qb + qt:
                                    continue
                                last_kt = 4 * qb + qt if variant == "causal" else NST - 1
                                nc.tensor.matmul(
                                    augs[qt][:],
                                    texp[:, qt * P : (qt + 1) * P],
                                    vaug[:, kt, h, :],
                                    start=(kt == 0),
                                    stop=(kt == last_kt),
                                )
                        for qt in range(4):
                            qt_g = qb * 4 + qt
                            rec = smallp.tile([P, 1], F32, tag="rec", name="rec")
                            nc.vector.reciprocal(rec[:], augs[qt][:, HD : HD + 1])
                            attn_n = npool.tile([P, HD], CDT, tag="attn_n", name="attn_n")
                            nc.vector.tensor_scalar_mul(
                                attn_n[:], augs[qt][:, 0:HD], rec[:]
                            )
                            ps_t = pspool.tile([P, P], CDT, tag="ps", name="tr")
                            nc.tensor.transpose(ps_t[:], attn_n[:], tid[:])
                            nc.scalar.copy(
                                attnT[:, h, qt_g * P : (qt_g + 1) * P], ps_t[:]
                            )

              if dump:
                  nc.sync.dma_start(d_attnT.ap(), attnT[:])

              # ---------------- out projection ----------------
              with tc.tile_pool(name="outp", bufs=4) as outp:
                for st in range(NST):
                    for eb in range(NEB):
                        ps_o = pspool.tile([P, EB], F32, tag="ps", name=f"o{st}_{eb}")
                        for dc in range(HG):
                            nc.tensor.matmul(
                                ps_o[:],
                                attnT[:, dc, st * P : (st + 1) * P],
                                wo_sb[:, dc, eb * EB : (eb + 1) * EB],
                                start=(dc == 0),
                                stop=(dc == HG - 1),
                            )
                        out_t = outp.tile([P, EB], F32, tag="outsb", name="outsb")
                        nc.scalar.copy(out_t[:], ps_o[:])
                        nc.sync.dma_start(
                            y[st * P : (st + 1) * P, eb * EB : (eb + 1) * EB], out_t[:]
                        )

    nc.compile()
    return nc


_PROGRAM_CACHE: dict[str, object] = {}
_last_in_maps = None


def _get_program(variant: str):
    key = f"{variant}:{COMPUTE_DTYPE}"
    if key not in _PROGRAM_CACHE:
        _PROGRAM_CACHE[key] = build_program(variant)
    return _PROGRAM_CACHE[key]


def _detect_variant(mask: np.ndarray) -> str:
    if not np.any(mask):
        return "none"
    causal = np.triu(np.full((S, S), NEG, dtype=np.float32), 1)
    if np.array_equal(mask, causal):
        return "causal"
    return "general"


def _np_cdt():
    if COMPUTE_DTYPE == "bfloat16":
        import ml_dtypes

        return ml_dtypes.bfloat16
    return np.float32


def make_in_maps(x, wq, wk, wv, wo, cos, sin, mask, variant):
    npdt = _np_cdt()
    cosT = np.repeat(cos.T, 2, axis=0)  # [HD, S]
    sinT = np.repeat(sin.T, 2, axis=0)
    sinT = sinT.copy()
    sinT[0::2, :] *= -1.0  # row 2i holds -sin, row 2i+1 holds +sin
    shared = {
        "cosT": np.ascontiguousarray(cosT).astype(npdt),
        "sinT": np.ascontiguousarray(sinT).astype(npdt),
        "ident": np.eye(P, dtype=np.float32).astype(npdt),
    }
    if variant == "causal":
        # scoresT layout is [kp, q]: masked where kp > q -> strict lower triangle
        shared["tri"] = np.tril(np.full((P, P), NEG, dtype=np.float32), -1).astype(npdt)
    elif variant == "general":
        shared["maskT"] = np.ascontiguousarray(mask.T * math.sqrt(HD)).astype(npdt)

    xTs = [np.ascontiguousarray(x[b].T).astype(npdt) for b in range(B)]
    in_maps = []
    for core in range(NCORES):
        b, g = divmod(core, NCORES // B)
        sl = slice(g * DG, (g + 1) * DG)
        in_maps.append(
            {
                "xT": xTs[b],
                "wq": np.ascontiguousarray(wq[:, sl]).astype(npdt),
                "wk": np.ascontiguousarray(wk[:, sl]).astype(npdt),
                "wv": np.ascontiguousarray(wv[:, sl]).astype(npdt),
                "wo": np.ascontiguousarray(wo[sl, :]).astype(npdt),
                **shared,
            }
        )
    return in_maps


def kernel(x, wq, wk, wv, wo, cos, sin, mask):
    x = np.asarray(x, dtype=np.float32)
    wq = np.asarray(wq, dtype=np.float32)
    wk = np.asarray(wk, dtype=np.float32)
    wv = np.asarray(wv, dtype=np.float32)
    wo = np.asarray(wo, dtype=np.float32)
    cos = np.asarray(cos, dtype=np.float32)
    sin = np.asarray(sin, dtype=np.float32)
    mask = np.asarray(mask, dtype=np.float32)

    variant = _detect_variant(mask)
    nc = _get_program(variant)
    in_maps = make_in_maps(x, wq, wk, wv, wo, cos, sin, mask, variant)

    global _last_in_maps
    _last_in_maps = in_maps

    res = run_bass_kernel_spmd(nc, in_maps, core_ids=list(range(NCORES)))

    out = np.empty((B, S, D), dtype=np.float32)
    gpb = NCORES // B
    for b in range(B):
        acc = np.zeros((S, D), dtype=np.float64)
        for g in range(gpb):
            acc += res.results[b * gpb + g]["y"].astype(np.float64)
        out[b] = acc.astype(np.float32)
    return out


# revision 18
# speedup vs baseline: 1.2727x; 1.2189x over previous
"""Fused multi-head attention (QKV proj + RoPE + causal softmax + out proj)
for Trainium2, sharded over 8 NeuronCores.

Sharding: data-parallel over batch (B=2) x tensor-parallel over heads
(16 heads -> 4 per core).  Each core computes, for its (batch, head-group):
  qT/kT = wq/wk^T-projections in [d, s] layout (CDT matmuls, fp32 PSUM)
  RoPE applied on-chip (DVE pair-swap via stream_shuffle + mul/add)
  scoresT[kp, q] = krot^T.T @ qrot (one K=128 matmul per tile)
  causal masking via a PE-accumulated triangular constant on diagonal tiles
  expT = exp(scale * scoresT) on ACT
  PV with a ones-augmented V column => unnormalized out + softmax denominator
  normalize (DVE reciprocal + tensor_scalar), PE-transpose to attnT[d, s]
  partial output y_g = attnT.T @ wo_rows  (summed over head-groups on host)

Inputs arrive full-size; host slices/transposes, feeds 8 SPMD cores, and
sums the 4 head-group partials per batch at the end.
"""

import math

import numpy as np

import concourse.bacc as bacc
import concourse.mybir as mybir
from concourse import tile
from concourse.bass_utils import run_bass_kernel_spmd

B, S, D, H = 2, 2048, 2048, 16
NCORES = 8
HG = 4  # heads per core
HD = D // H  # 128
DG = HG * HD  # 512 = per-core slice of D
P = 128
NKC = D // P  # 16 contraction chunks
SBLK = 512  # s-block width in projection passes
NSB = S // SBLK
NST = S // P  # 16 s-tiles of 128
QB = 512  # q-block width in attention
NQB = S // QB
EB = 512  # e-block width in out-projection
NEB = D // EB

F32 = mybir.dt.float32
EXP = mybir.ActivationFunctionType.Exp
SCALE = 1.0 / math.sqrt(HD)
SWAP32 = [i ^ 1 for i in range(32)]
NEG = -1.0e9

COMPUTE_DTYPE = "bfloat16"  # or "bfloat16"


def build_program(variant: str, dump: bool = False, cdt_name: str | None = None):
    """variant: 'causal' | 'none' | 'general'"""
    CDT = getattr(mybir.dt, cdt_name or COMPUTE_DTYPE)
    nc = bacc.Bacc("TRN2", target_bir_lowering=False, debug=False)
    xT = nc.dram_tensor("xT", [D, S], CDT, kind="ExternalInput")
    wq = nc.dram_tensor("wq", [D, DG], CDT, kind="ExternalInput")
    wk = nc.dram_tensor("wk", [D, DG], CDT, kind="ExternalInput")
    wv = nc.dram_tensor("wv", [D, DG], CDT, kind="ExternalInput")
    wo = nc.dram_tensor("wo", [DG, D], CDT, kind="ExternalInput")
    cosT = nc.dram_tensor("cosT", [HD, S], CDT, kind="ExternalInput")
    sinT = nc.dram_tensor("sinT", [HD, S], CDT, kind="ExternalInput")
    ident = nc.dram_tensor("ident", [P, P], CDT, kind="ExternalInput")
    tri = None
    maskT = None
    if variant == "causal":
        tri = nc.dram_tensor("tri", [P, P], CDT, kind="ExternalInput")
    elif variant == "general":
        # mask.T pre-scaled by sqrt(HD) on host so exp's scale recovers it
        maskT = nc.dram_tensor("maskT", [S, S], CDT, kind="ExternalInput")
    y = nc.dram_tensor("y", [S, D], F32, kind="ExternalOutput")
    d_qrot = d_krot = d_vaug = d_attnT = None
    if dump:
        d_qrot = nc.dram_tensor("d_qrot", [P, HG, S], CDT, kind="ExternalOutput")
        d_krot = nc.dram_tensor("d_krot", [P, HG, S], CDT, kind="ExternalOutput")
        d_vaug = nc.dram_tensor("d_vaug", [P, NST, HG, HD + 2], CDT, kind="ExternalOutput")
        d_attnT = nc.dram_tensor("d_attnT", [P, HG, S], CDT, kind="ExternalOutput")

    with tile.TileContext(nc) as tc:
        with (
            tc.tile_pool(name="const", bufs=1) as constp,
            tc.tile_pool(name="big", bufs=1) as bigp,
            # one PSUM pool shared by every phase: no pool-scoping barriers,
            # so attention matmuls can start while the v-pass drains.
            tc.tile_pool(name="ps", bufs=8, space="PSUM") as pspool,
        ):
            tid = constp.tile([P, P], CDT)
            nc.sync.dma_start(tid[:], ident[:])
            ttri = None
            if variant == "causal":
                ttri = constp.tile([P, P], CDT)
                nc.sync.dma_start(ttri[:], tri[:])
            tcos = constp.tile([HD, S], CDT)
            nc.sync.dma_start(tcos[:], cosT[:])
            tsin = constp.tile([HD, S], CDT)
            nc.sync.dma_start(tsin[:], sinT[:])

            qrot = bigp.tile([P, HG, S], CDT, tag="qrot")
            krot = bigp.tile([P, HG, S], CDT, tag="krot")
            vaug = bigp.tile([P, NST, HG, HD + 2], CDT, tag="vaug")
            ones_view = vaug[:, :, :, HD : HD + 2]
            if CDT == mybir.dt.float32r:
                nc.vector.memset(ones_view.bitcast(F32), 1.0)
            else:
                nc.vector.memset(ones_view, 1.0)

            # ---------------- projections + RoPE ----------------
            # Per-kc weight tiles stream ahead of the matmuls; xT row-block
            # tiles are consumed by the in-flight accumulations then die.
            with (
                tc.tile_pool(name="wpool", bufs=12) as wpool,
                tc.tile_pool(name="xpool", bufs=8) as xpool,
                tc.tile_pool(name="rope", bufs=3) as ropep,
            ):
                for proj, wdram in (("q", wq), ("k", wk), ("v", wv)):
                    for sb in range(NSB):
                        nun = SBLK // P if proj == "v" else HG
                        pss = [
                            pspool.tile(
                                [P, SBLK if proj != "v" else DG],
                                F32,
                                tag="ps",
                                name=f"ps_{proj}_{sb}_{u}",
                            )
                            for u in range(nun)
                        ]
                        for kc in range(NKC):
                            xt = xpool.tile([P, SBLK], CDT, tag="xt", name="xt")
                            nc.sync.dma_start(
                                xt[:],
                                xT[kc * P : (kc + 1) * P, sb * SBLK : (sb + 1) * SBLK],
                            )
                            wt = wpool.tile([P, DG], CDT, tag="wt", name="wt")
                            nc.sync.dma_start(wt[:], wdram[kc * P : (kc + 1) * P, :])
                            if proj in ("q", "k"):
                                for dt in range(HG):
                                    nc.tensor.matmul(
                                        pss[dt][:],
                                        wt[:, dt * HD : (dt + 1) * HD],
                                        xt[:],
                                        start=(kc == 0),
                                        stop=(kc == NKC - 1),
                                    )
                            else:
                                for st in range(SBLK // P):
                                    nc.tensor.matmul(
                                        pss[st][:],
                                        xt[:, st * P : (st + 1) * P],
                                        wt[:],
                                        start=(kc == 0),
                                        stop=(kc == NKC - 1),
                                    )
                        if proj in ("q", "k"):
                            dstbuf = qrot if proj == "q" else krot
                            ssl = slice(sb * SBLK, (sb + 1) * SBLK)
                            for dt in range(HG):
                                ps = pss[dt]
                                tsw = ropep.tile([P, SBLK], F32, tag="tsw", name="tsw")
                                nc.vector.stream_shuffle(tsw[:], ps[:], SWAP32)
                                t1 = ropep.tile([P, SBLK], F32, tag="t1", name="t1")
                                nc.vector.tensor_mul(t1[:], ps[:], tcos[:, ssl])
                                t2 = ropep.tile([P, SBLK], F32, tag="t2", name="t2")
                                nc.vector.tensor_mul(t2[:], tsw[:], tsin[:, ssl])
                                nc.vector.tensor_add(
                                    dstbuf[:, dt, ssl], t1[:], t2[:]
                                )
                        else:
                            for st in range(SBLK // P):
                                st_g = sb * (SBLK // P) + st
                                for h in range(HG):
                                    nc.scalar.copy(
                                        vaug[:, st_g, h, 0:HD],
                                        pss[st][:, h * HD : (h + 1) * HD],
                                    )

            if dump:
                nc.sync.dma_start(d_qrot.ap(), qrot[:])
                nc.sync.dma_start(d_krot.ap(), krot[:])
                nc.sync.dma_start(d_vaug.ap(), vaug[:])

            # ---------------- attention (+ wo weights prefetch) ----------------
            with (
                tc.tile_pool(name="attn_out", bufs=1) as atp,
                tc.tile_pool(name="wopool", bufs=1) as wopool,
            ):
              attnT = atp.tile([P, HG, S], CDT, tag="attnT")
              wo_sb = wopool.tile([P, HG, D], CDT, tag="wo")
              nc.sync.dma_start(
                  wo_sb[:], wo.ap().rearrange("(dc p) e -> p dc e", p=P)
              )
              with (
                tc.tile_pool(name="mask", bufs=2) as maskp,
                tc.tile_pool(name="expp", bufs=4) as epool,
                tc.tile_pool(name="small", bufs=4) as smallp,
                tc.tile_pool(name="normp", bufs=3) as npool,
              ):
                for qb in range(NQB):
                    mts = None
                    if variant == "general":
                        mts = maskp.tile([P, NST, QB], CDT, tag="mt", name="mt")
                        nc.sync.dma_start(
                            mts[:],
                            maskT[:, qb * QB : (qb + 1) * QB].rearrange(
                                "(kt p) q -> p kt q", p=P
                            ),
                        )
                    nkt = 4 * (qb + 1) if variant == "causal" else NST
                    for h in range(HG):
                        augs = [
                            pspool.tile([P, HD + 2], F32, tag="ps", name=f"aug{i}")
                            for i in range(4)
                        ]
                        for kt in range(nkt):
                            ps_s = pspool.tile([P, QB], F32, tag="ps", name="scores")
                            if variant == "causal" and kt >= 4 * qb:
                                j = kt - 4 * qb
                                nc.tensor.matmul(
                                    ps_s[:, j * P : QB],
                                    krot[:, h, kt * P : (kt + 1) * P],
                                    qrot[:, h, qb * QB + j * P : (qb + 1) * QB],
                                    start=True,
                                    stop=False,
                                )
                                nc.tensor.matmul(
                                    ps_s[:, j * P : (j + 1) * P],
                                    tid[:],
                                    ttri[:],
                                    start=False,
                                    stop=True,
                                )
                                valid = slice(j * P, QB)
                                qt_lo = j
                            else:
                                last = variant != "general"
                                nc.tensor.matmul(
                                    ps_s[:],
                                    krot[:, h, kt * P : (kt + 1) * P],
                                    qrot[:, h, qb * QB : (qb + 1) * QB],
                                    start=True,
                                    stop=last,
                                )
                                if variant == "general":
                                    nc.tensor.matmul(
                                        ps_s[:],
                                        tid[:],
                                        mts[:, kt, :],
                                        start=False,
                                        stop=True,
                                    )
                                valid = slice(0, QB)
                                qt_lo = 0
                            texp = epool.tile([P, QB], CDT, tag="exp", name="exp")
                            nc.scalar.activation(
                                texp[:, valid], ps_s[:, valid], EXP, scale=SCALE
                            )
                            for qt in range(qt_lo, 4):
                                if variant == "causal" and kt > 4 * qb + qt:
                                    continue
                                last_kt = 4 * qb + qt if variant == "causal" else NST - 1
                                nc.tensor.matmul(
                                    augs[qt][:],
                                    texp[:, qt * P : (qt + 1) * P],
                                    vaug[:, kt, h, :],
                                    start=(kt == 0),
                                    stop=(kt == last_kt),
                                )
                        for qt in range(4):
                            qt_g = qb * 4 + qt
                            rec = smallp.tile([P, 1], F32, tag="rec", name="rec")
                            nc.vector.reciprocal(rec[:], augs[qt][:, HD : HD + 1])
                            attn_n = npool.tile([P, HD], CDT, tag="attn_n", name="attn_n")
                            nc.vector.tensor_scalar_mul(
                                attn_n[:], augs[qt][:, 0:HD], rec[:]
                            )
                            ps_t = pspool.tile([P, P], CDT, tag="ps", name="tr")
                            nc.tensor.transpose(ps_t[:], attn_n[:], tid[:])
                            nc.scalar.copy(
                                attnT[:, h, qt_g * P : (qt_g + 1) * P], ps_t[:]
                            )

              if dump:
                  nc.sync.dma_start(d_attnT.ap(), attnT[:])

              # ---------------- out projection ----------------
              with tc.tile_pool(name="outp", bufs=4) as outp:
                for st in range(NST):
                    for eb in range(NEB):
                        ps_o = pspool.tile([P, EB], F32, tag="ps", name=f"o{st}_{eb}")
                        for dc in range(HG):
                            nc.tensor.matmul(
                                ps_o[:],
                                attnT[:, dc, st * P : (st + 1) * P],
                                wo_sb[:, dc, eb * EB : (eb + 1) * EB],
                                start=(dc == 0),
                                stop=(dc == HG - 1),
                            )
                        out_t = outp.tile([P, EB], F32, tag="outsb", name="outsb")
                        nc.scalar.copy(out_t[:], ps_o[:])
                        nc.sync.dma_start(
                            y[st * P : (st + 1) * P, eb * EB : (eb + 1) * EB], out_t[:]
                        )

    nc.compile()
    return nc


_PROGRAM_CACHE: dict[str, object] = {}
_last_in_maps = None


def _get_program(variant: str):
    key = f"{variant}:{COMPUTE_DTYPE}"
    if key not in _PROGRAM_CACHE:
        _PROGRAM_CACHE[key] = build_program(variant)
    return _PROGRAM_CACHE[key]


def _detect_variant(mask: np.ndarray) -> str:
    if not np.any(mask):
        return "none"
    causal = np.triu(np.full((S, S), NEG, dtype=np.float32), 1)
    if np.array_equal(mask, causal):
        return "causal"
    return "general"


def _np_cdt():
    if COMPUTE_DTYPE == "bfloat16":
        import ml_dtypes

        return ml_dtypes.bfloat16
    return np.float32


def make_in_maps(x, wq, wk, wv, wo, cos, sin, mask, variant):
    npdt = _np_cdt()
    cosT = np.repeat(cos.T, 2, axis=0)  # [HD, S]
    sinT = np.repeat(sin.T, 2, axis=0)
    sinT = sinT.copy()
    sinT[0::2, :] *= -1.0  # row 2i holds -sin, row 2i+1 holds +sin
    shared = {
        "cosT": np.ascontiguousarray(cosT).astype(npdt),
        "sinT": np.ascontiguousarray(sinT).astype(npdt),
        "ident": np.eye(P, dtype=np.float32).astype(npdt),
    }
    if variant == "causal":
        # scoresT layout is [kp, q]: masked where kp > q -> strict lower triangle
        shared["tri"] = np.tril(np.full((P, P), NEG, dtype=np.float32), -1).astype(npdt)
    elif variant == "general":
        shared["maskT"] = np.ascontiguousarray(mask.T * math.sqrt(HD)).astype(npdt)

    xTs = [np.ascontiguousarray(x[b].T).astype(npdt) for b in range(B)]
    in_maps = []
    for core in range(NCORES):
        b, g = divmod(core, NCORES // B)
        sl = slice(g * DG, (g + 1) * DG)
        in_maps.append(
            {
                "xT": xTs[b],
                "wq": np.ascontiguousarray(wq[:, sl]).astype(npdt),
                "wk": np.ascontiguousarray(wk[:, sl]).astype(npdt),
                "wv": np.ascontiguousarray(wv[:, sl]).astype(npdt),
                "wo": np.ascontiguousarray(wo[sl, :]).astype(npdt),
                **shared,
            }
        )
    return in_maps


def kernel(x, wq, wk, wv, wo, cos, sin, mask):
    x = np.asarray(x, dtype=np.float32)
    wq = np.asarray(wq, dtype=np.float32)
    wk = np.asarray(wk, dtype=np.float32)
    wv = np.asarray(wv, dtype=np.float32)
    wo = np.asarray(wo, dtype=np.float32)
    cos = np.asarray(cos, dtype=np.float32)
    sin = np.asarray(sin, dtype=np.float32)
    mask = np.asarray(mask, dtype=np.float32)

    variant = _detect_variant(mask)
    nc = _get_program(variant)
    in_maps = make_in_maps(x, wq, wk, wv, wo, cos, sin, mask, variant)

    global _last_in_maps
    _last_in_maps = in_maps

    res = run_bass_kernel_spmd(nc, in_maps, core_ids=list(range(NCORES)))

    out = np.empty((B, S, D), dtype=np.float32)
    gpb = NCORES // B
    for b in range(B):
        acc = np.zeros((S, D), dtype=np.float64)
        for g in range(gpb):
            acc += res.results[b * gpb + g]["y"].astype(np.float64)
        out[b] = acc.astype(np.float32)
    return out


# revision 23
# speedup vs baseline: 1.3205x; 1.0375x over previous
"""Fused multi-head attention (QKV proj + RoPE + causal softmax + out proj)
for Trainium2, sharded over 8 NeuronCores.

Sharding: data-parallel over batch (B=2) x tensor-parallel over heads
(16 heads -> 4 per core).  Each core computes, for its (batch, head-group):
  qT/kT = wq/wk^T-projections in [d, s] layout (CDT matmuls, fp32 PSUM)
  RoPE applied on-chip (DVE pair-swap via stream_shuffle + mul/add)
  scoresT[kp, q] = krot^T.T @ qrot (one K=128 matmul per tile)
  causal masking via a PE-accumulated triangular constant on diagonal tiles
  expT = exp(scale * scoresT) on ACT
  PV with a ones-augmented V column => unnormalized out + softmax denominator
  normalize (DVE reciprocal + tensor_scalar), PE-transpose to attnT[d, s]
  partial output y_g = attnT.T @ wo_rows  (summed over head-groups on host)

Inputs arrive full-size; host slices/transposes, feeds 8 SPMD cores, and
sums the 4 head-group partials per batch at the end.
"""

import math

import numpy as np

import concourse.bacc as bacc
import concourse.mybir as mybir
from concourse import tile
from concourse.bass_utils import run_bass_kernel_spmd

B, S, D, H = 2, 2048, 2048, 16
NCORES = 8
HG = 4  # heads per core
HD = D // H  # 128
DG = HG * HD  # 512 = per-core slice of D
P = 128
NKC = D // P  # 16 contraction chunks
SBLK = 512  # s-block width in projection passes
NSB = S // SBLK
NST = S // P  # 16 s-tiles of 128
QB = 512  # q-block width in attention
NQB = S // QB
EB = 512  # e-block width in out-projection
NEB = D // EB

F32 = mybir.dt.float32
EXP = mybir.ActivationFunctionType.Exp
SCALE = 1.0 / math.sqrt(HD)
SWAP32 = [i ^ 1 for i in range(32)]
NEG = -1.0e9

COMPUTE_DTYPE = "bfloat16"  # or "bfloat16"


def build_program(variant: str, dump: bool = False, cdt_name: str | None = None):
    """variant: 'causal' | 'none' | 'general'"""
    CDT = getattr(mybir.dt, cdt_name or COMPUTE_DTYPE)
    nc = bacc.Bacc("TRN2", target_bir_lowering=False, debug=False)
    xT = nc.dram_tensor("xT", [D, S], CDT, kind="ExternalInput")
    wq = nc.dram_tensor("wq", [D, DG], CDT, kind="ExternalInput")
    wk = nc.dram_tensor("wk", [D, DG], CDT, kind="ExternalInput")
    wv = nc.dram_tensor("wv", [D, DG], CDT, kind="ExternalInput")
    wo = nc.dram_tensor("wo", [DG, D], CDT, kind="ExternalInput")
    cosT = nc.dram_tensor("cosT", [HD, S], CDT, kind="ExternalInput")
    sinT = nc.dram_tensor("sinT", [HD, S], CDT, kind="ExternalInput")
    ident = nc.dram_tensor("ident", [P, P], CDT, kind="ExternalInput")
    tri = None
    maskT = None
    if variant == "causal":
        tri = nc.dram_tensor("tri", [P, P], CDT, kind="ExternalInput")
    elif variant == "general":
        # mask.T pre-scaled by sqrt(HD) on host so exp's scale recovers it
        maskT = nc.dram_tensor("maskT", [S, S], CDT, kind="ExternalInput")
    y = nc.dram_tensor("y", [S, D], F32, kind="ExternalOutput")
    d_qrot = d_krot = d_vaug = d_attnT = None
    if dump:
        d_qrot = nc.dram_tensor("d_qrot", [P, HG, S], CDT, kind="ExternalOutput")
        d_krot = nc.dram_tensor("d_krot", [P, HG, S], CDT, kind="ExternalOutput")
        d_vaug = nc.dram_tensor("d_vaug", [P, NST, HG, HD + 2], CDT, kind="ExternalOutput")
        d_attnT = nc.dram_tensor("d_attnT", [P, HG, S], CDT, kind="ExternalOutput")

    with tile.TileContext(nc) as tc:
        with (
            tc.tile_pool(name="const", bufs=1) as constp,
            tc.tile_pool(name="big", bufs=1) as bigp,
            # one PSUM pool shared by every phase: no pool-scoping barriers,
            # so attention matmuls can start while the v-pass drains.
            tc.tile_pool(name="ps", bufs=8, space="PSUM") as pspool,
        ):
            tid = constp.tile([P, P], CDT)
            nc.sync.dma_start(tid[:], ident[:])
            ttri = None
            if variant == "causal":
                ttri = constp.tile([P, P], CDT)
                nc.sync.dma_start(ttri[:], tri[:])
            tcos = constp.tile([HD, S], CDT)
            nc.sync.dma_start(tcos[:], cosT[:])
            tsin = constp.tile([HD, S], CDT)
            nc.sync.dma_start(tsin[:], sinT[:])

            qrot = bigp.tile([P, HG, S], CDT, tag="qrot")
            krot = bigp.tile([P, HG, S], CDT, tag="krot")
            vaug = bigp.tile([P, NST, HG, HD + 2], CDT, tag="vaug")
            ones_view = vaug[:, :, :, HD : HD + 2]
            if CDT == mybir.dt.float32r:
                nc.vector.memset(ones_view.bitcast(F32), 1.0)
            else:
                nc.vector.memset(ones_view, 1.0)

            # ---------------- projections + RoPE ----------------
            # Per-kc weight tiles stream ahead of the matmuls; xT row-block
            # tiles are consumed by the in-flight accumulations then die.
            with (
                tc.tile_pool(name="wpool", bufs=12) as wpool,
                tc.tile_pool(name="xpool", bufs=8) as xpool,
                tc.tile_pool(name="rope", bufs=3) as ropep,
            ):
                for proj, wdram in (("q", wq), ("k", wk), ("v", wv)):
                    for sb in range(NSB):
                        nun = SBLK // P if proj == "v" else HG
                        pss = [
                            pspool.tile(
                                [P, SBLK if proj != "v" else DG],
                                F32,
                                tag="ps",
                                name=f"ps_{proj}_{sb}_{u}",
                            )
                            for u in range(nun)
                        ]
                        for kc in range(NKC):
                            xt = xpool.tile([P, SBLK], CDT, tag="xt", name="xt")
                            nc.gpsimd.dma_start(
                                xt[:],
                                xT[kc * P : (kc + 1) * P, sb * SBLK : (sb + 1) * SBLK],
                            )
                            wt = wpool.tile([P, DG], CDT, tag="wt", name="wt")
                            nc.sync.dma_start(wt[:], wdram[kc * P : (kc + 1) * P, :])
                            if proj in ("q", "k"):
                                for dt in range(HG):
                                    nc.tensor.matmul(
                                        pss[dt][:],
                                        wt[:, dt * HD : (dt + 1) * HD],
                                        xt[:],
                                        start=(kc == 0),
                                        stop=(kc == NKC - 1),
                                    )
                            else:
                                for st in range(SBLK // P):
                                    nc.tensor.matmul(
                                        pss[st][:],
                                        xt[:, st * P : (st + 1) * P],
                                        wt[:],
                                        start=(kc == 0),
                                        stop=(kc == NKC - 1),
                                    )
                        if proj in ("q", "k"):
                            dstbuf = qrot if proj == "q" else krot
                            ssl = slice(sb * SBLK, (sb + 1) * SBLK)
                            for dt in range(HG):
                                # fast ACT copy frees the PSUM bank; DVE RoPE
                                # then runs from SBUF at bf16 2x rates
                                qsb = ropep.tile([P, SBLK], CDT, tag="qsb", name="qsb")
                                nc.scalar.copy(qsb[:], pss[dt][:])
                                tsw = ropep.tile([P, SBLK], CDT, tag="tsw", name="tsw")
                                nc.vector.stream_shuffle(tsw[:], qsb[:], SWAP32)
                                t1 = ropep.tile([P, SBLK], CDT, tag="t1", name="t1")
                                nc.vector.tensor_mul(t1[:], qsb[:], tcos[:, ssl])
                                t2 = ropep.tile([P, SBLK], CDT, tag="t2", name="t2")
                                nc.vector.tensor_mul(t2[:], tsw[:], tsin[:, ssl])
                                nc.vector.tensor_add(
                                    dstbuf[:, dt, ssl], t1[:], t2[:]
                                )
                        else:
                            for st in range(SBLK // P):
                                st_g = sb * (SBLK // P) + st
                                for h in range(HG):
                                    nc.scalar.copy(
                                        vaug[:, st_g, h, 0:HD],
                                        pss[st][:, h * HD : (h + 1) * HD],
                                    )

            if dump:
                nc.sync.dma_start(d_qrot.ap(), qrot[:])
                nc.sync.dma_start(d_krot.ap(), krot[:])
                nc.sync.dma_start(d_vaug.ap(), vaug[:])

            # ---------------- attention (+ wo weights prefetch) ----------------
            with (
                tc.tile_pool(name="attn_out", bufs=1) as atp,
                tc.tile_pool(name="wopool", bufs=1) as wopool,
            ):
              attnT = atp.tile([P, HG, S], CDT, tag="attnT")
              wo_sb = wopool.tile([P, HG, D], CDT, tag="wo")
              nc.sync.dma_start(
                  wo_sb[:], wo.ap().rearrange("(dc p) e -> p dc e", p=P)
              )
              with (
                tc.tile_pool(name="mask", bufs=2) as maskp,
                tc.tile_pool(name="expp", bufs=4) as epool,
                tc.tile_pool(name="small", bufs=4) as smallp,
                tc.tile_pool(name="normp", bufs=3) as npool,
                tc.tile_pool(name="outp", bufs=4) as outp,
              ):
                for qb in range(NQB):
                    mts = None
                    if variant == "general":
                        mts = maskp.tile([P, NST, QB], CDT, tag="mt", name="mt")
                        nc.sync.dma_start(
                            mts[:],
                            maskT[:, qb * QB : (qb + 1) * QB].rearrange(
                                "(kt p) q -> p kt q", p=P
                            ),
                        )
                    nkt = 4 * (qb + 1) if variant == "causal" else NST
                    for h in range(HG):
                        augs = [
                            pspool.tile([P, HD + 2], F32, tag="ps", name=f"aug{i}")
                            for i in range(4)
                        ]
                        for kt in range(nkt):
                            ps_s = pspool.tile([P, QB], F32, tag="ps", name="scores")
                            if variant == "causal" and kt >= 4 * qb:
                                j = kt - 4 * qb
                                nc.tensor.matmul(
                                    ps_s[:, j * P : QB],
                                    krot[:, h, kt * P : (kt + 1) * P],
                                    qrot[:, h, qb * QB + j * P : (qb + 1) * QB],
                                    start=True,
                                    stop=False,
                                )
                                nc.tensor.matmul(
                                    ps_s[:, j * P : (j + 1) * P],
                                    tid[:],
                                    ttri[:],
                                    start=False,
                                    stop=True,
                                )
                                valid = slice(j * P, QB)
                                qt_lo = j
                            else:
                                last = variant != "general"
                                nc.tensor.matmul(
                                    ps_s[:],
                                    krot[:, h, kt * P : (kt + 1) * P],
                                    qrot[:, h, qb * QB : (qb + 1) * QB],
                                    start=True,
                                    stop=last,
                                )
                                if variant == "general":
                                    nc.tensor.matmul(
                                        ps_s[:],
                                        tid[:],
                                        mts[:, kt, :],
                                        start=False,
                                        stop=True,
                                    )
                                valid = slice(0, QB)
                                qt_lo = 0
                            texp = epool.tile([P, QB], CDT, tag="exp", name="exp")
                            nc.scalar.activation(
                                texp[:, valid], ps_s[:, valid], EXP, scale=SCALE
                            )
                            for qt in range(qt_lo, 4):
                                if variant == "causal" and kt > 4 * qb + qt:
                                    continue
                                last_kt = 4 * qb + qt if variant == "causal" else NST - 1
                                nc.tensor.matmul(
                                    augs[qt][:],
                                    texp[:, qt * P : (qt + 1) * P],
                                    vaug[:, kt, h, :],
                                    start=(kt == 0),
                                    stop=(kt == last_kt),
                                )
                        for qt in range(4):
                            qt_g = qb * 4 + qt
                            # fast ACT copy releases the aug PSUM bank early
                            aug_sb = npool.tile(
                                [P, HD + 2], F32, tag="aug_sb", name="aug_sb"
                            )
                            nc.scalar.copy(aug_sb[:], augs[qt][:])
                            rec = smallp.tile([P, 1], F32, tag="rec", name="rec")
                            nc.vector.reciprocal(rec[:], aug_sb[:, HD : HD + 1])
                            attn_n = npool.tile([P, HD], CDT, tag="attn_n", name="attn_n")
                            nc.vector.tensor_scalar_mul(
                                attn_n[:], aug_sb[:, 0:HD], rec[:]
                            )
                            ps_t = pspool.tile([P, P], CDT, tag="ps", name="tr")
                            nc.tensor.transpose(ps_t[:], attn_n[:], tid[:])
                            nc.scalar.copy(
                                attnT[:, h, qt_g * P : (qt_g + 1) * P], ps_t[:]
                            )

                    # interleaved out projection for this qb's s-range:
                    # keeps PE dense (HAM warm) and removes the tail phase
                    for st in range(qb * 4, qb * 4 + 4):
                        for eb in range(NEB):
                            ps_o = pspool.tile(
                                [P, EB], F32, tag="ps", name=f"o{st}_{eb}"
                            )
                            for dc in range(HG):
                                nc.tensor.matmul(
                                    ps_o[:],
                                    attnT[:, dc, st * P : (st + 1) * P],
                                    wo_sb[:, dc, eb * EB : (eb + 1) * EB],
                                    start=(dc == 0),
                                    stop=(dc == HG - 1),
                                )
                            out_t = outp.tile([P, EB], F32, tag="outsb", name="outsb")
                            nc.scalar.copy(out_t[:], ps_o[:])
                            nc.sync.dma_start(
                                y[st * P : (st + 1) * P, eb * EB : (eb + 1) * EB],
                                out_t[:],
                            )

              if dump:
                  nc.sync.dma_start(d_attnT.ap(), attnT[:])

    nc.compile()
    return nc


_PROGRAM_CACHE: dict[str, object] = {}
_last_in_maps = None


def _get_program(variant: str):
    key = f"{variant}:{COMPUTE_DTYPE}"
    if key not in _PROGRAM_CACHE:
        _PROGRAM_CACHE[key] = build_program(variant)
    return _PROGRAM_CACHE[key]


def _detect_variant(mask: np.ndarray) -> str:
    if not np.any(mask):
        return "none"
    causal = np.triu(np.full((S, S), NEG, dtype=np.float32), 1)
    if np.array_equal(mask, causal):
        return "causal"
    return "general"


def _np_cdt():
    if COMPUTE_DTYPE == "bfloat16":
        import ml_dtypes

        return ml_dtypes.bfloat16
    return np.float32


def make_in_maps(x, wq, wk, wv, wo, cos, sin, mask, variant):
    npdt = _np_cdt()
    cosT = np.repeat(cos.T, 2, axis=0)  # [HD, S]
    sinT = np.repeat(sin.T, 2, axis=0)
    sinT = sinT.copy()
    sinT[0::2, :] *= -1.0  # row 2i holds -sin, row 2i+1 holds +sin
    shared = {
        "cosT": np.ascontiguousarray(cosT).astype(npdt),
        "sinT": np.ascontiguousarray(sinT).astype(npdt),
        "ident": np.eye(P, dtype=np.float32).astype(npdt),
    }
    if variant == "causal":
        # scoresT layout is [kp, q]: masked where kp > q -> strict lower triangle
        shared["tri"] = np.tril(np.full((P, P), NEG, dtype=np.float32), -1).astype(npdt)
    elif variant == "general":
        shared["maskT"] = np.ascontiguousarray(mask.T * math.sqrt(HD)).astype(npdt)

    xTs = [np.ascontiguousarray(x[b].T).astype(npdt) for b in range(B)]
    in_maps = []
    for core in range(NCORES):
        b, g = divmod(core, NCORES // B)
        sl = slice(g * DG, (g + 1) * DG)
        in_maps.append(
            {
                "xT": xTs[b],
                "wq": np.ascontiguousarray(wq[:, sl]).astype(npdt),
                "wk": np.ascontiguousarray(wk[:, sl]).astype(npdt),
                "wv": np.ascontiguousarray(wv[:, sl]).astype(npdt),
                "wo": np.ascontiguousarray(wo[sl, :]).astype(npdt),
                **shared,
            }
        )
    return in_maps


def kernel(x, wq, wk, wv, wo, cos, sin, mask):
    x = np.asarray(x, dtype=np.float32)
    wq = np.asarray(wq, dtype=np.float32)
    wk = np.asarray(wk, dtype=np.float32)
    wv = np.asarray(wv, dtype=np.float32)
    wo = np.asarray(wo, dtype=np.float32)
    cos = np.asarray(cos, dtype=np.float32)
    sin = np.asarray(sin, dtype=np.float32)
    mask = np.asarray(mask, dtype=np.float32)

    variant = _detect_variant(mask)
    nc = _get_program(variant)
    in_maps = make_in_maps(x, wq, wk, wv, wo, cos, sin, mask, variant)

    global _last_in_maps
    _last_in_maps = in_maps

    res = run_bass_kernel_spmd(nc, in_maps, core_ids=list(range(NCORES)))

    out = np.empty((B, S, D), dtype=np.float32)
    gpb = NCORES // B
    for b in range(B):
        acc = np.zeros((S, D), dtype=np.float64)
        for g in range(gpb):
            acc += res.results[b * gpb + g]["y"].astype(np.float64)
        out[b] = acc.astype(np.float32)
    return out


# revision 24
# speedup vs baseline: 1.5488x; 1.1729x over previous
"""Fused multi-head attention (QKV proj + RoPE + causal softmax + out proj)
for Trainium2, sharded over 8 NeuronCores.

Sharding: data-parallel over batch (B=2) x tensor-parallel over heads
(16 heads -> 4 per core).  Each core computes, for its (batch, head-group):
  qT/kT = wq/wk^T-projections in [d, s] layout (CDT matmuls, fp32 PSUM)
  RoPE applied on-chip (DVE pair-swap via stream_shuffle + mul/add)
  scoresT[kp, q] = krot^T.T @ qrot (one K=128 matmul per tile)
  causal masking via a PE-accumulated triangular constant on diagonal tiles
  expT = exp(scale * scoresT) on ACT
  PV with a ones-augmented V column => unnormalized out + softmax denominator
  normalize (DVE reciprocal + tensor_scalar), PE-transpose to attnT[d, s]
  partial output y_g = attnT.T @ wo_rows  (summed over head-groups on host)

Inputs arrive full-size; host slices/transposes, feeds 8 SPMD cores, and
sums the 4 head-group partials per batch at the end.
"""

import math

import numpy as np

import concourse.bacc as bacc
import concourse.mybir as mybir
from concourse import tile
from concourse.bass_utils import run_bass_kernel_spmd

B, S, D, H = 2, 2048, 2048, 16
NCORES = 8
HG = 4  # heads per core
HD = D // H  # 128
DG = HG * HD  # 512 = per-core slice of D
P = 128
NKC = D // P  # 16 contraction chunks
SBLK = 512  # s-block width in projection passes
NSB = S // SBLK
NST = S // P  # 16 s-tiles of 128
QB = 512  # q-block width in attention
NQB = S // QB
EB = 512  # e-block width in out-projection
NEB = D // EB

F32 = mybir.dt.float32
EXP = mybir.ActivationFunctionType.Exp
SCALE = 1.0 / math.sqrt(HD)
SWAP32 = [i ^ 1 for i in range(32)]
NEG = -1.0e9

COMPUTE_DTYPE = "bfloat16"  # or "bfloat16"


def build_program(variant: str, dump: bool = False, cdt_name: str | None = None):
    """variant: 'causal' | 'none' | 'general'"""
    CDT = getattr(mybir.dt, cdt_name or COMPUTE_DTYPE)
    nc = bacc.Bacc("TRN2", target_bir_lowering=False, debug=False)
    xT = nc.dram_tensor("xT", [D, S], CDT, kind="ExternalInput")
    wq = nc.dram_tensor("wq", [D, DG], CDT, kind="ExternalInput")
    wk = nc.dram_tensor("wk", [D, DG], CDT, kind="ExternalInput")
    wv = nc.dram_tensor("wv", [D, DG], CDT, kind="ExternalInput")
    wo = nc.dram_tensor("wo", [DG, D], CDT, kind="ExternalInput")
    cosT = nc.dram_tensor("cosT", [HD, S], CDT, kind="ExternalInput")
    sinT = nc.dram_tensor("sinT", [HD, S], CDT, kind="ExternalInput")
    ident = nc.dram_tensor("ident", [P, P], CDT, kind="ExternalInput")
    tri = None
    maskT = None
    if variant == "causal":
        tri = nc.dram_tensor("tri", [P, P], CDT, kind="ExternalInput")
    elif variant == "general":
        # mask.T pre-scaled by sqrt(HD) on host so exp's scale recovers it
        maskT = nc.dram_tensor("maskT", [S, S], CDT, kind="ExternalInput")
    y = nc.dram_tensor("y", [S, D], F32, kind="ExternalOutput")
    d_qrot = d_krot = d_vaug = d_attnT = None
    if dump:
        d_qrot = nc.dram_tensor("d_qrot", [P, HG, S], CDT, kind="ExternalOutput")
        d_krot = nc.dram_tensor("d_krot", [P, HG, S], CDT, kind="ExternalOutput")
        d_vaug = nc.dram_tensor("d_vaug", [P, NST, HG, HD + 2], CDT, kind="ExternalOutput")
        d_attnT = nc.dram_tensor("d_attnT", [P, HG, S], CDT, kind="ExternalOutput")

    with tile.TileContext(nc) as tc:
        with (
            tc.tile_pool(name="const", bufs=1) as constp,
            tc.tile_pool(name="big", bufs=1) as bigp,
            # one PSUM pool shared by every phase: no pool-scoping barriers,
            # so attention matmuls can start while the v-pass drains.
            tc.tile_pool(name="ps", bufs=6, space="PSUM") as pspool,
            tc.tile_pool(name="psaux", bufs=2, space="PSUM") as psaux,
        ):
            tid = constp.tile([P, P], CDT)
            nc.sync.dma_start(tid[:], ident[:])
            ttri = None
            if variant == "causal":
                ttri = constp.tile([P, P], CDT)
                nc.sync.dma_start(ttri[:], tri[:])
            tcos = constp.tile([HD, S], CDT)
            nc.sync.dma_start(tcos[:], cosT[:])
            tsin = constp.tile([HD, S], CDT)
            nc.sync.dma_start(tsin[:], sinT[:])

            qrot = bigp.tile([P, HG, S], CDT, tag="qrot")
            krot = bigp.tile([P, HG, S], CDT, tag="krot")
            vaug = bigp.tile([P, NST, HG, HD + 2], CDT, tag="vaug")
            ones_view = vaug[:, :, :, HD : HD + 2]
            if CDT == mybir.dt.float32r:
                nc.vector.memset(ones_view.bitcast(F32), 1.0)
            else:
                nc.vector.memset(ones_view, 1.0)

            # ---------------- projections + RoPE ----------------
            # Per-kc weight tiles stream ahead of the matmuls; xT row-block
            # tiles are consumed by the in-flight accumulations then die.
            with (
                tc.tile_pool(name="wpool", bufs=12) as wpool,
                tc.tile_pool(name="xpool", bufs=8) as xpool,
                tc.tile_pool(name="rope", bufs=3) as ropep,
            ):
                for proj, wdram in (("q", wq), ("k", wk), ("v", wv)):
                    for sb in range(NSB):
                        nun = SBLK // P if proj == "v" else HG
                        pss = [
                            pspool.tile(
                                [P, SBLK if proj != "v" else DG],
                                F32,
                                tag="ps",
                                name=f"ps_{proj}_{sb}_{u}",
                            )
                            for u in range(nun)
                        ]
                        for kc in range(NKC):
                            xt = xpool.tile([P, SBLK], CDT, tag="xt", name="xt")
                            nc.gpsimd.dma_start(
                                xt[:],
                                xT[kc * P : (kc + 1) * P, sb * SBLK : (sb + 1) * SBLK],
                            )
                            wt = wpool.tile([P, DG], CDT, tag="wt", name="wt")
                            nc.sync.dma_start(wt[:], wdram[kc * P : (kc + 1) * P, :])
                            if proj in ("q", "k"):
                                for dt in range(HG):
                                    nc.tensor.matmul(
                                        pss[dt][:],
                                        wt[:, dt * HD : (dt + 1) * HD],
                                        xt[:],
                                        start=(kc == 0),
                                        stop=(kc == NKC - 1),
                                    )
                            else:
                                for st in range(SBLK // P):
                                    nc.tensor.matmul(
                                        pss[st][:],
                                        xt[:, st * P : (st + 1) * P],
                                        wt[:],
                                        start=(kc == 0),
                                        stop=(kc == NKC - 1),
                                    )
                        if proj in ("q", "k"):
                            dstbuf = qrot if proj == "q" else krot
                            ssl = slice(sb * SBLK, (sb + 1) * SBLK)
                            for dt in range(HG):
                                # fast ACT copy frees the PSUM bank; DVE RoPE
                                # then runs from SBUF at bf16 2x rates
                                qsb = ropep.tile([P, SBLK], CDT, tag="qsb", name="qsb")
                                nc.vector.tensor_copy(qsb[:], pss[dt][:])
                                tsw = ropep.tile([P, SBLK], CDT, tag="tsw", name="tsw")
                                nc.vector.stream_shuffle(tsw[:], qsb[:], SWAP32)
                                t1 = ropep.tile([P, SBLK], CDT, tag="t1", name="t1")
                                nc.vector.tensor_mul(t1[:], qsb[:], tcos[:, ssl])
                                t2 = ropep.tile([P, SBLK], CDT, tag="t2", name="t2")
                                nc.vector.tensor_mul(t2[:], tsw[:], tsin[:, ssl])
                                nc.vector.tensor_add(
                                    dstbuf[:, dt, ssl], t1[:], t2[:]
                                )
                        else:
                            for st in range(SBLK // P):
                                st_g = sb * (SBLK // P) + st
                                for h in range(HG):
                                    nc.vector.tensor_copy(
                                        vaug[:, st_g, h, 0:HD],
                                        pss[st][:, h * HD : (h + 1) * HD],
                                    )

            if dump:
                nc.sync.dma_start(d_qrot.ap(), qrot[:])
                nc.sync.dma_start(d_krot.ap(), krot[:])
                nc.sync.dma_start(d_vaug.ap(), vaug[:])

            # ---------------- attention (+ wo weights prefetch) ----------------
            with (
                tc.tile_pool(name="attn_out", bufs=1) as atp,
                tc.tile_pool(name="wopool", bufs=1) as wopool,
            ):
              attnT = atp.tile([P, HG, S], CDT, tag="attnT")
              wo_sb = wopool.tile([P, HG, D], CDT, tag="wo")
              nc.sync.dma_start(
                  wo_sb[:], wo.ap().rearrange("(dc p) e -> p dc e", p=P)
              )
              with (
                tc.tile_pool(name="mask", bufs=2) as maskp,
                tc.tile_pool(name="expp", bufs=4) as epool,
                tc.tile_pool(name="small", bufs=4) as smallp,
                tc.tile_pool(name="normp", bufs=3) as npool,
                tc.tile_pool(name="outp", bufs=4) as outp,
              ):
                for qb in range(NQB):
                    mts = None
                    if variant == "general":
                        mts = maskp.tile([P, NST, QB], CDT, tag="mt", name="mt")
                        nc.sync.dma_start(
                            mts[:],
                            maskT[:, qb * QB : (qb + 1) * QB].rearrange(
                                "(kt p) q -> p kt q", p=P
                            ),
                        )
                    nkt = 4 * (qb + 1) if variant == "causal" else NST
                    for h in range(HG):
                        augs = [
                            pspool.tile([P, HD + 2], F32, tag="ps", name=f"aug{i}")
                            for i in range(4)
                        ]
                        for kt in range(nkt):
                            ps_s = pspool.tile([P, QB], F32, tag="ps", name="scores")
                            if variant == "causal" and kt >= 4 * qb:
                                j = kt - 4 * qb
                                nc.tensor.matmul(
                                    ps_s[:, j * P : QB],
                                    krot[:, h, kt * P : (kt + 1) * P],
                                    qrot[:, h, qb * QB + j * P : (qb + 1) * QB],
                                    start=True,
                                    stop=False,
                                )
                                nc.tensor.matmul(
                                    ps_s[:, j * P : (j + 1) * P],
                                    tid[:],
                                    ttri[:],
                                    start=False,
                                    stop=True,
                                )
                                valid = slice(j * P, QB)
                                qt_lo = j
                            else:
                                last = variant != "general"
                                nc.tensor.matmul(
                                    ps_s[:],
                                    krot[:, h, kt * P : (kt + 1) * P],
                                    qrot[:, h, qb * QB : (qb + 1) * QB],
                                    start=True,
                                    stop=last,
                                )
                                if variant == "general":
                                    nc.tensor.matmul(
                                        ps_s[:],
                                        tid[:],
                                        mts[:, kt, :],
                                        start=False,
                                        stop=True,
                                    )
                                valid = slice(0, QB)
                                qt_lo = 0
                            texp = epool.tile([P, QB], CDT, tag="exp", name="exp")
                            nc.scalar.activation(
                                texp[:, valid], ps_s[:, valid], EXP, scale=SCALE
                            )
                            for qt in range(qt_lo, 4):
                                if variant == "causal" and kt > 4 * qb + qt:
                                    continue
                                last_kt = 4 * qb + qt if variant == "causal" else NST - 1
                                nc.tensor.matmul(
                                    augs[qt][:],
                                    texp[:, qt * P : (qt + 1) * P],
                                    vaug[:, kt, h, :],
                                    start=(kt == 0),
                                    stop=(kt == last_kt),
                                )
                                if kt == last_kt:
                                    # normalize + transpose immediately: frees
                                    # the aug bank and keeps PE/DVE interleaved
                                    qt_g = qb * 4 + qt
                                    aug_sb = npool.tile(
                                        [P, HD + 2], F32, tag="aug_sb", name="aug_sb"
                                    )
                                    nc.vector.tensor_copy(aug_sb[:], augs[qt][:])
                                    rec = smallp.tile([P, 1], F32, tag="rec", name="rec")
                                    nc.vector.reciprocal(
                                        rec[:], aug_sb[:, HD : HD + 1]
                                    )
                                    attn_n = npool.tile(
                                        [P, HD], CDT, tag="attn_n", name="attn_n"
                                    )
                                    nc.vector.tensor_scalar_mul(
                                        attn_n[:], aug_sb[:, 0:HD], rec[:]
                                    )
                                    ps_t = psaux.tile([P, P], CDT, tag="tr", name="tr")
                                    nc.tensor.transpose(ps_t[:], attn_n[:], tid[:])
                                    nc.vector.tensor_copy(
                                        attnT[:, h, qt_g * P : (qt_g + 1) * P],
                                        ps_t[:],
                                    )

                    # interleaved out projection for this qb's s-range:
                    # keeps PE dense (HAM warm) and removes the tail phase
                    for st in range(qb * 4, qb * 4 + 4):
                        for eb in range(NEB):
                            ps_o = psaux.tile(
                                [P, EB], F32, tag="tr", name=f"o{st}_{eb}"
                            )
                            for dc in range(HG):
                                nc.tensor.matmul(
                                    ps_o[:],
                                    attnT[:, dc, st * P : (st + 1) * P],
                                    wo_sb[:, dc, eb * EB : (eb + 1) * EB],
                                    start=(dc == 0),
                                    stop=(dc == HG - 1),
                                )
                            out_t = outp.tile([P, EB], F32, tag="outsb", name="outsb")
                            nc.vector.tensor_copy(out_t[:], ps_o[:])
                            nc.sync.dma_start(
                                y[st * P : (st + 1) * P, eb * EB : (eb + 1) * EB],
                                out_t[:],
                            )

              if dump:
                  nc.sync.dma_start(d_attnT.ap(), attnT[:])

    nc.compile()
    return nc


_PROGRAM_CACHE: dict[str, object] = {}
_last_in_maps = None


def _get_program(variant: str):
    key = f"{variant}:{COMPUTE_DTYPE}"
    if key not in _PROGRAM_CACHE:
        _PROGRAM_CACHE[key] = build_program(variant)
    return _PROGRAM_CACHE[key]


def _detect_variant(mask: np.ndarray) -> str:
    if not np.any(mask):
        return "none"
    causal = np.triu(np.full((S, S), NEG, dtype=np.float32), 1)
    if np.array_equal(mask, causal):
        return "causal"
    return "general"


def _np_cdt():
    if COMPUTE_DTYPE == "bfloat16":
        import ml_dtypes

        return ml_dtypes.bfloat16
    return np.float32


def make_in_maps(x, wq, wk, wv, wo, cos, sin, mask, variant):
    npdt = _np_cdt()
    cosT = np.repeat(cos.T, 2, axis=0)  # [HD, S]
    sinT = np.repeat(sin.T, 2, axis=0)
    sinT = sinT.copy()
    sinT[0::2, :] *= -1.0  # row 2i holds -sin, row 2i+1 holds +sin
    shared = {
        "cosT": np.ascontiguousarray(cosT).astype(npdt),
        "sinT": np.ascontiguousarray(sinT).astype(npdt),
        "ident": np.eye(P, dtype=np.float32).astype(npdt),
    }
    if variant == "causal":
        # scoresT layout is [kp, q]: masked where kp > q -> strict lower triangle
        shared["tri"] = np.tril(np.full((P, P), NEG, dtype=np.float32), -1).astype(npdt)
    elif variant == "general":
        shared["maskT"] = np.ascontiguousarray(mask.T * math.sqrt(HD)).astype(npdt)

    xTs = [np.ascontiguousarray(x[b].T).astype(npdt) for b in range(B)]
    in_maps = []
    for core in range(NCORES):
        b, g = divmod(core, NCORES // B)
        sl = slice(g * DG, (g + 1) * DG)
        in_maps.append(
            {
                "xT": xTs[b],
                "wq": np.ascontiguousarray(wq[:, sl]).astype(npdt),
                "wk": np.ascontiguousarray(wk[:, sl]).astype(npdt),
                "wv": np.ascontiguousarray(wv[:, sl]).astype(npdt),
                "wo": np.ascontiguousarray(wo[sl, :]).astype(npdt),
                **shared,
            }
        )
    return in_maps


def kernel(x, wq, wk, wv, wo, cos, sin, mask):
    x = np.asarray(x, dtype=np.float32)
    wq = np.asarray(wq, dtype=np.float32)
    wk = np.asarray(wk, dtype=np.float32)
    wv = np.asarray(wv, dtype=np.float32)
    wo = np.asarray(wo, dtype=np.float32)
    cos = np.asarray(cos, dtype=np.float32)
    sin = np.asarray(sin, dtype=np.float32)
    mask = np.asarray(mask, dtype=np.float32)

    variant = _detect_variant(mask)
    nc = _get_program(variant)
    in_maps = make_in_maps(x, wq, wk, wv, wo, cos, sin, mask, variant)

    global _last_in_maps
    _last_in_maps = in_maps

    res = run_bass_kernel_spmd(nc, in_maps, core_ids=list(range(NCORES)))

    out = np.empty((B, S, D), dtype=np.float32)
    gpb = NCORES // B
    for b in range(B):
        acc = np.zeros((S, D), dtype=np.float64)
        for g in range(gpb):
            acc += res.results[b * gpb + g]["y"].astype(np.float64)
        out[b] = acc.astype(np.float32)
    return out
